# revision 1
# baseline (speedup 1.0000x reference)
"""Trainium2 Bass kernel for the snntorch-style 2-layer spiking net.

Reference semantics (per time step t, B batch, fp32):
    cur1 = x[:,t,:] @ W1.T + b1              # [B,128]
    mem1 = 0.9*mem1 + cur1 - spk1_prev       # reset-by-subtract (TH=1)
    spk1 = (mem1 > 1)
    cur2 = spk1 @ W2.T + b2                  # [B,10]
    mem2 = 0.9*mem2 + cur2 - spk2_prev
    spk2 = (mem2 > 1)
    outputs: spk2_rec, mem2_rec each [T, B, 10]

Distribution: pure data parallel over 8 NeuronCores (B=2048 -> 256/core).

Numerics: the spiking dynamics are chaotic (threshold crossings), so the
layer-1 matmul must be fp32-grade. Native fp32 matmul on the PE runs at 1/4
rate, so x and W1 are split into bf16 hi/lo pairs and the product is computed
with three bf16 passes (hi*hi + hi*lo + lo*hi), which matches fp32 accuracy
(verified: same spike-flip count as a pure-f32 reduction-order change).

Per-core layout (h = hidden on partitions for layer 1, o = output on
partitions for layer 2, batch on the free dim):
  - x shipped host-transposed as [784, T, 256] bf16 hi and lo planes; three
    all-ones rows appended to the hi plane carry the folded bias (see below).
  - P1[t] (psum) accumulates 19 matmuls: 6 hi*W1h + 6 hi*W1l + 6 lo*W1h full
    K=128 chunks + 1 merged remainder (K=51: 16 hi-rem rows + 3 bias rows +
    16 hi-rem rows again + 16 lo-rem rows) + the reset matmul -0.5*I @ sign1.
  - spk1 is never materialized: sign1 = Sign(mem1-1) in {-1,1} (bf16, exact),
    with spk1 = 0.5*sign1 + 0.5 folded into weights/biases:
      b1_eff = b1 - 0.5 (reset constant), reset weight -0.5*I,
      cur2 = (0.5*W2) @ sign1 + (b2 + 0.5*W2.sum(1)).
  - biases enter psum via matmul against constant all-ones rows, split into
    3 bf16 components each so the folded constants keep fp32 precision.
  - recurrence: one DVE scalar_tensor_tensor per layer per step
    (mem' = mem*0.9 + PSUM), sign1 on the scalar engine, spk2 via DVE is_gt.

Outputs per core: mem2 history [10, T, 256] f32 and spk2 history [10, T, 256]
bf16 (0/1 exact); host transposes/gathers to [T, 2048, 10].
"""
import contextlib

import numpy as np
import ml_dtypes

import concourse.bass as bass
import concourse.tile as tile
from concourse import bacc, mybir
from concourse import bass_utils

N_CORES = 8
B, T, NIN, NH, NOUT = 2048, 201, 784, 128, 10
BS = B // N_CORES          # batch per core = 256
TB = 8                     # time-block (input DMA / output granularity)
NFULL = 6                  # full K=128 chunks (6*128=768)
REM = NIN - NFULL * 128    # 16 remainder rows
KREM = REM + 3 + REM + REM # merged remainder contraction: hi+bias3, hi, lo
BETA = 0.9
THR = 1.0

BF16 = ml_dtypes.bfloat16


def _split3_f64(v):
    """Split float64 vector into 3 bf16 components summing to ~2^-27 accuracy."""
    h = v.astype(BF16)
    r = v - h.astype(np.float64)
    m = r.astype(BF16)
    r2 = r - m.astype(np.float64)
    l = r2.astype(BF16)
    return h, m, l


def build_kernel(reps_loop=False):
    """Build the SPMD Bass program (one core's view; all cores identical).

    reps_loop=True wraps the body in a dynamic For_i driven by the "reps"
    input so test.py can measure HW time by wall-clock differencing.
    """
    nc = bacc.Bacc("TRN2", target_bir_lowering=False, debug=False,
                   num_devices=N_CORES)

    xh = nc.dram_tensor("xh", [NIN + 3, T, BS], mybir.dt.bfloat16,
                        kind="ExternalInput").ap()
    xl = nc.dram_tensor("xl", [NIN, T, BS], mybir.dt.bfloat16,
                        kind="ExternalInput").ap()
    wh = nc.dram_tensor("wh", [128, NFULL, NH], mybir.dt.bfloat16,
                        kind="ExternalInput").ap()
    wl = nc.dram_tensor("wl", [128, NFULL, NH], mybir.dt.bfloat16,
                        kind="ExternalInput").ap()
    wrem = nc.dram_tensor("wrem", [KREM, NH], mybir.dt.bfloat16,
                          kind="ExternalInput").ap()
    w2 = nc.dram_tensor("w2", [NH, 2, NOUT], mybir.dt.bfloat16,
                        kind="ExternalInput").ap()
    b2e = nc.dram_tensor("b2e", [NOUT, 1], mybir.dt.float32,
                         kind="ExternalInput").ap()
    m2out = nc.dram_tensor("m2out", [NOUT, T, BS], mybir.dt.float32,
                           kind="ExternalOutput").ap()
    s2out = nc.dram_tensor("s2out", [NOUT, T, BS], mybir.dt.bfloat16,
                           kind="ExternalOutput").ap()
    if reps_loop:
        reps = nc.dram_tensor("reps", [1, 1], mybir.dt.int32,
                              kind="ExternalInput").ap()

    blocks = []
    t0 = 0
    while t0 < T:
        tb = min(TB, T - t0)
        blocks.append((t0, tb))
        t0 += tb
    t2b = {}
    for bi, (bt0, btb) in enumerate(blocks):
        for ti in range(btb):
            t2b[bt0 + ti] = (bi, ti)

    with tile.TileContext(nc) as tc:
        with tc.tile_pool(name="wpool", bufs=1) as wpool, \
             tc.tile_pool(name="xpool", bufs=3) as xpool, \
             tc.tile_pool(name="state", bufs=1) as state, \
             tc.tile_pool(name="hist", bufs=2) as hist, \
             tc.tile_pool(name="p1pool", bufs=4, space="PSUM") as p1pool, \
             tc.tile_pool(name="p2pool", bufs=2, space="PSUM") as p2pool:

            # ---- constant weights (loaded once) ----
            wh_t = wpool.tile([128, NFULL, NH], mybir.dt.bfloat16)
            nc.sync.dma_start(wh_t[:], wh[:])
            wl_t = wpool.tile([128, NFULL, NH], mybir.dt.bfloat16)
            nc.sync.dma_start(wl_t[:], wl[:])
            wrem_t = wpool.tile([KREM, NH], mybir.dt.bfloat16)
            nc.sync.dma_start(wrem_t[:], wrem[:])
            w2_t = wpool.tile([NH, 2, NOUT], mybir.dt.bfloat16)
            nc.sync.dma_start(w2_t[:], w2[:])
            b2e_t = wpool.tile([NOUT, 1], mybir.dt.float32)
            nc.sync.dma_start(b2e_t[:], b2e[:])
            biasm1 = wpool.tile([NH, 1], mybir.dt.float32)
            nc.gpsimd.memset(biasm1[:], -THR)

            if reps_loop:
                rt = wpool.tile([1, 1], mybir.dt.int32)
                nc.sync.dma_start(rt[:], reps[:])
                regs = []
                for eng in (nc.tensor, nc.vector, nc.scalar, nc.gpsimd, nc.sync):
                    r = eng.alloc_register(f"reps_{len(regs)}")
                    eng.reg_load(r, rt[0:1, 0:1])
                    regs.append(r)
                rv = nc.snap(bass.RegisterHandles(regs), min_val=0,
                             max_val=1 << 20)
                loop_cm = tc.For_i(0, rv, 1)
            else:
                loop_cm = contextlib.nullcontext()

            with loop_cm:
                # ---- initial state ----
                sign1_init = state.tile([NH, BS], mybir.dt.bfloat16)
                nc.gpsimd.memset(sign1_init[:], -1.0)   # spk1_prev = 0
                mem1_init = state.tile([NH, BS], mybir.dt.float32)
                nc.gpsimd.memset(mem1_init[:], 0.0)
                m2_init = state.tile([NOUT, BS], mybir.dt.float32)
                nc.gpsimd.memset(m2_init[:], 0.0)
                s2_init = state.tile([NOUT, BS], mybir.dt.bfloat16)
                nc.gpsimd.memset(s2_init[:], 0.0)  # spk2_prev = 0

                # ---- per-block input tiles, DMA'd ahead ----
                xh_tiles = [None] * len(blocks)
                xl_tiles = [None] * len(blocks)
                rem_tiles = [None] * len(blocks)

                pending_dmas = []

                def load_block(bi, defer=False):
                    bt0, btb = blocks[bi]
                    xh_b = xpool.tile([128, NFULL, btb, BS], mybir.dt.bfloat16,
                                      name=f"xh_b{bi}", tag="xh_b")
                    xl_b = xpool.tile([128, NFULL, btb, BS], mybir.dt.bfloat16,
                                      name=f"xl_b{bi}", tag="xl_b")
                    rem_b = xpool.tile([KREM, btb, BS], mybir.dt.bfloat16,
                                       name=f"rem_b{bi}", tag="rem_b")
                    thunks = []
                    for c in range(NFULL):
                        thunks.append(lambda c=c: nc.sync.dma_start(
                            xh_b[:, c, :, :],
                            xh[c * 128:(c + 1) * 128, bt0:bt0 + btb, :]))
                        thunks.append(lambda c=c: nc.sync.dma_start(
                            xl_b[:, c, :, :],
                            xl[c * 128:(c + 1) * 128, bt0:bt0 + btb, :]))
                    # rem rows: xh remainder + 3 ones rows (baked into xh),
                    # xh remainder again (W1l pass), xl remainder (W1h pass)
                    thunks.append(lambda: nc.sync.dma_start(
                        rem_b[0:REM + 3, :, :],
                        xh[NFULL * 128:, bt0:bt0 + btb, :]))
                    thunks.append(lambda: nc.sync.dma_start(
                        rem_b[REM + 3:2 * REM + 3, :, :],
                        xh[NFULL * 128:NFULL * 128 + REM, bt0:bt0 + btb, :]))
                    thunks.append(lambda: nc.sync.dma_start(
                        rem_b[2 * REM + 3:, :, :],
                        xl[NFULL * 128:, bt0:bt0 + btb, :]))
                    xh_tiles[bi] = xh_b
                    xl_tiles[bi] = xl_b
                    rem_tiles[bi] = rem_b
                    if defer:
                        pending_dmas.extend(thunks)
                    else:
                        for th in thunks:
                            th()

                def chunks_mms(t, p1):
                    """The 19 recurrence-independent matmuls for step t."""
                    bi, ti = t2b[t]
                    xh_b, xl_b, rem_b = xh_tiles[bi], xl_tiles[bi], rem_tiles[bi]
                    nc.tensor.matmul(p1[:], wrem_t[:], rem_b[:, ti, :],
                                     start=True, stop=False)
                    for c in range(NFULL):
                        nc.tensor.matmul(p1[:], wh_t[:, c, :], xh_b[:, c, ti, :],
                                         start=False, stop=False)
                    for c in range(NFULL):
                        nc.tensor.matmul(p1[:], wl_t[:, c, :], xh_b[:, c, ti, :],
                                         start=False, stop=False)
                    for c in range(NFULL):
                        nc.tensor.matmul(p1[:], wh_t[:, c, :], xl_b[:, c, ti, :],
                                         start=False, stop=(c == NFULL - 1))

                # ---- software-pipelined main loop ----
                load_block(0)
                load_block(1)

                LOOKAHEAD = 2
                p1_tiles = {}
                for t in range(LOOKAHEAD):
                    p1_tiles[t] = p1pool.tile([NH, BS], mybir.dt.float32,
                                              name=f"p1_{t}", tag="p1")
                    chunks_mms(t, p1_tiles[t])

                sign1_prev = sign1_init
                mem1_prev = mem1_init
                m2hist_prev, m2pcol = m2_init, 0      # tile + col index of mem2(t-1)
                s2hist_prev, s2pcol = s2_init, 0      # tile + col of spk2(t-1)
                m2hist = s2hist = None

                for t in range(T):
                    bi, ti = t2b[t]
                    bt0, btb = blocks[bi]

                    if ti == 0:
                        # new block: allocate output history tiles
                        m2hist = hist.tile([NOUT, btb * BS], mybir.dt.float32,
                                           name=f"m2h_{bi}", tag="m2h")
                        s2hist = hist.tile([NOUT, btb * BS],
                                           mybir.dt.bfloat16,
                                           name=f"s2h_{bi}", tag="s2h")
                        # prefetch a future block's inputs, DMAs spread
                        # across this block's iterations
                        if bi + 2 < len(blocks):
                            load_block(bi + 2, defer=True)

                    # drain a couple of deferred prefetch DMAs per step
                    for _ in range(2):
                        if pending_dmas:
                            pending_dmas.pop(0)()

                    p1 = p1_tiles.pop(t)
                    # u1 = -0.5*sign1(t-1) + P1(t)      (DVE, psum operand)
                    u1 = state.tile([NH, BS], mybir.dt.float32,
                                    name=f"u1_{t % 2}", tag="u1", bufs=2)
                    nc.vector.scalar_tensor_tensor(
                        u1[:], sign1_prev[:], -0.5, p1[:],
                        mybir.AluOpType.mult, mybir.AluOpType.add)
                    # mem1(t) = 0.9*mem1(t-1) + u1
                    mem1 = state.tile([NH, BS], mybir.dt.float32,
                                      name=f"mem1_{t % 2}", tag="mem1", bufs=2)
                    nc.vector.scalar_tensor_tensor(
                        mem1[:], mem1_prev[:], BETA, u1[:],
                        mybir.AluOpType.mult, mybir.AluOpType.add)

                    # sign1(t) = Sign(mem1 - 1)  (ACT, bf16 out)
                    sign1 = state.tile([NH, BS], mybir.dt.bfloat16,
                                       name=f"sign1_{t % 3}", tag="sign1", bufs=3)
                    nc.scalar.sign(sign1[:], mem1[:], bias=biasm1[:])

                    # keep TE busy while DVE/ACT run: stream future chunks
                    if t + LOOKAHEAD < T:
                        p1n = p1pool.tile([NH, BS], mybir.dt.float32,
                                          name=f"p1_{t + LOOKAHEAD}", tag="p1")
                        p1_tiles[t + LOOKAHEAD] = p1n
                        chunks_mms(t + LOOKAHEAD, p1n)

                    # layer 2: P2 = 0.5*W2@sign1 (hi+lo)
                    p2 = p2pool.tile([NOUT, BS], mybir.dt.float32,
                                     name=f"p2_{t % 2}", tag="p2")
                    nc.tensor.matmul(p2[:], w2_t[:, 0, :], sign1[:],
                                     start=True, stop=False)
                    nc.tensor.matmul(p2[:], w2_t[:, 1, :], sign1[:],
                                     start=False, stop=True)

                    # u2 = spk2(t-1) - b2_eff - P2      (DVE, psum operand)
                    u2 = state.tile([NOUT, BS], mybir.dt.float32,
                                    name=f"u2_{t % 2}", tag="u2", bufs=2)
                    nc.vector.scalar_tensor_tensor(
                        u2[:], s2hist_prev[:, s2pcol * BS:(s2pcol + 1) * BS],
                        b2e_t[:, 0:1], p2[:],
                        mybir.AluOpType.subtract, mybir.AluOpType.subtract)
                    # mem2(t) = 0.9*mem2(t-1) - u2 -> written into history col
                    m2dst = m2hist[:, ti * BS:(ti + 1) * BS]
                    nc.vector.scalar_tensor_tensor(
                        m2dst, m2hist_prev[:, m2pcol * BS:(m2pcol + 1) * BS],
                        BETA, u2[:],
                        mybir.AluOpType.mult, mybir.AluOpType.subtract)
                    # spk2(t) = mem2 > 1 (bf16 0/1) -> history col
                    nc.vector.tensor_scalar(
                        s2hist[0:NOUT, ti * BS:(ti + 1) * BS], m2dst, THR, None,
                        mybir.AluOpType.is_gt)

                    mem1_prev = mem1
                    sign1_prev = sign1
                    m2hist_prev, m2pcol = m2hist, ti
                    s2hist_prev, s2pcol = s2hist, ti

                    if ti == btb - 1:
                        # block done: store outputs
                        nc.sync.dma_start(
                            m2out[:, bt0:bt0 + btb, :],
                            m2hist[:].rearrange("o (t b) -> o t b", t=btb))
                        nc.sync.dma_start(
                            s2out[:, bt0:bt0 + btb, :],
                            s2hist[:].rearrange("o (t b) -> o t b", t=btb))

    nc.compile()
    return nc


def prepare_inputs(x, W1, b1, W2, b2):
    """Host-side sharding + dtype splitting. Returns in_maps for 8 cores."""
    x = np.ascontiguousarray(x, dtype=np.float32)
    W1 = np.asarray(W1, dtype=np.float32)
    b1 = np.asarray(b1, dtype=np.float32)
    W2 = np.asarray(W2, dtype=np.float32)
    b2 = np.asarray(b2, dtype=np.float32)

    # hi/lo split of x (bf16), via bit ops (ml_dtypes' bf16->f32 cast is slow)
    u = x.view(np.uint32)
    r = (u + np.uint32(0x7FFF) + ((u >> np.uint32(16)) & np.uint32(1))) \
        & np.uint32(0xFFFF0000)          # round-to-nearest-even to bf16
    xh16 = (r >> np.uint32(16)).astype(np.uint16).view(BF16)
    xl16 = (x - r.view(np.float32)).astype(BF16)

    # W1 splits, transposed to [784, 128]
    W1h = W1.astype(BF16)
    W1l = (W1 - W1h.astype(np.float32)).astype(BF16)
    W1hT = np.ascontiguousarray(W1h.T)
    W1lT = np.ascontiguousarray(W1l.T)
    wh = np.ascontiguousarray(
        W1hT[:NFULL * 128].reshape(NFULL, 128, NH).transpose(1, 0, 2))
    wl = np.ascontiguousarray(
        W1lT[:NFULL * 128].reshape(NFULL, 128, NH).transpose(1, 0, 2))

    # merged remainder weights [KREM, 128]
    b1h, b1m, b1l = _split3_f64(b1.astype(np.float64) - 0.5)
    wrem = np.concatenate([
        W1hT[NFULL * 128:],
        b1h[None, :].astype(BF16), b1m[None, :].astype(BF16),
        b1l[None, :].astype(BF16),
        W1lT[NFULL * 128:],
        W1hT[NFULL * 128:],
    ], axis=0)
    assert wrem.shape == (KREM, NH)

    W2half = 0.5 * W2.astype(np.float64)        # exact (power of two)
    W2hi = W2half.astype(BF16)
    W2lo = (W2half - W2hi.astype(np.float64)).astype(BF16)
    w2 = np.stack([np.ascontiguousarray(W2hi.T), np.ascontiguousarray(W2lo.T)],
                  axis=1)                        # [128, 2, 10]

    b2eff = (b2.astype(np.float64) + W2half.sum(axis=1)).astype(np.float32)
    b2e = np.ascontiguousarray(b2eff[:, None])   # [10, 1] f32

    in_maps = []
    for c in range(N_CORES):
        sl = slice(c * BS, (c + 1) * BS)
        xh_c = np.empty((NIN + 3, T, BS), BF16)
        xh_c[:NIN] = xh16[sl].transpose(2, 1, 0)                  # [784,T,256]
        xh_c[NIN:] = 1.0
        xl_c = np.empty((NIN, T, BS), BF16)
        xl_c[:] = xl16[sl].transpose(2, 1, 0)
        in_maps.append({
            "xh": xh_c, "xl": xl_c, "wh": wh, "wl": wl, "wrem": wrem,
            "w2": w2, "b2e": b2e,
        })
    return in_maps


def postprocess(results):
    """Gather per-core outputs into (spk2_rec, mem2_rec) [T, B, 10] f32."""
    spk = np.empty((T, B, NOUT), np.float32)
    mem = np.empty((T, B, NOUT), np.float32)
    for c, r in enumerate(results):
        sl = slice(c * BS, (c + 1) * BS)
        mem[:, sl, :] = r["m2out"].transpose(1, 2, 0)
        spk[:, sl, :] = r["s2out"].astype(np.float32).transpose(1, 2, 0)
    return spk, mem


_NC_CACHE = {}


def kernel(x, W1, b1, W2, b2):
    if "nc" not in _NC_CACHE:
        _NC_CACHE["nc"] = build_kernel(reps_loop=False)
    nc = _NC_CACHE["nc"]
    in_maps = prepare_inputs(x, W1, b1, W2, b2)
    res = bass_utils.run_bass_kernel_spmd(
        nc, in_maps, core_ids=list(range(N_CORES)))
    return postprocess(res.results)



# revision 5
# speedup vs baseline: 1.7574x; 1.7574x over previous
"""Trainium2 Bass kernel for the snntorch-style 2-layer spiking net.

Reference semantics (per time step t, B batch, fp32):
    cur1 = x[:,t,:] @ W1.T + b1              # [B,128]
    mem1 = 0.9*mem1 + cur1 - spk1_prev       # reset-by-subtract (TH=1)
    spk1 = (mem1 > 1)
    cur2 = spk1 @ W2.T + b2                  # [B,10]
    mem2 = 0.9*mem2 + cur2 - spk2_prev
    spk2 = (mem2 > 1)
    outputs: spk2_rec, mem2_rec each [T, B, 10]

Distribution: pure data parallel over 8 NeuronCores (B=2048 -> 256/core).

Numerics: identical to the fp32-grade baseline — x and W1 split into bf16
hi/lo pairs, layer-1 matmul = three bf16 passes (hi*Wh + hi*Wl + lo*Wh)
accumulated in fp32 PSUM; sign trick for layer-1 spikes.

Performance structure (what changed vs the naive version):
  - Inputs packed host-side so each 8-step block needs 3 DMA instructions
    (xh plane, xl plane, merged remainder) with 4KB-contiguous descriptors,
    instead of 15 strided DMAs. DMA issue cost and HWDGE serialization drop
    ~5x; DMA hardware runs at the ~360GB/s roofline (input traffic
    161MB/core is the ridge floor alongside PE streaming).
  - Layer-1 reset (-0.5*sign1) is a PE matmul accumulated into the same
    PSUM bank as the input chunks, so the recurrence needs only ONE DVE op
    per step (mem1 = 0.9*mem1_prev + P1) instead of two; shortens the
    loop-carried chain to mem1(DVE) -> sign1(ACT) -> reset-mm(PE).
  - Output history DMAs issued from the Activation HWDGE ring to keep them
    off the SP sequencer.

Per-core layout: hidden (128) on partitions for layer 1, NOUT=10 on
partitions for layer 2, batch (256) on the free dim.

Outputs per core: mem2 history [10, T, 256] f32 and spk2 history [10, T, 256]
bf16 (0/1 exact); host transposes/gathers to [T, 2048, 10].
"""
import contextlib

import numpy as np
import ml_dtypes

import concourse.bass as bass
import concourse.tile as tile
from concourse import bacc, mybir
from concourse import bass_utils

N_CORES = 8
B, T, NIN, NH, NOUT = 2048, 201, 784, 128, 10
BS = B // N_CORES          # batch per core = 256
TB = 8                     # time-block (input DMA / output granularity)
NFULL = 6                  # full K=128 chunks (6*128=768)
REM = NIN - NFULL * 128    # 16 remainder rows
KREM = REM + 3 + REM + REM # merged remainder contraction: hi+bias3, hi, lo
BETA = 0.9
THR = 1.0

BF16 = ml_dtypes.bfloat16


def _split3_f64(v):
    """Split float64 vector into 3 bf16 components summing to ~2^-27 accuracy."""
    h = v.astype(BF16)
    r = v - h.astype(np.float64)
    m = r.astype(BF16)
    r2 = r - m.astype(np.float64)
    l = r2.astype(BF16)
    return h, m, l


def build_kernel(reps_loop=False):
    """Build the SPMD Bass program (one core's view; all cores identical).

    reps_loop=True wraps the body in a dynamic For_i driven by the "reps"
    input so test.py can measure HW time by wall-clock differencing.
    """
    nc = bacc.Bacc("TRN2", target_bir_lowering=False, debug=False,
                   num_devices=N_CORES)

    xh = nc.dram_tensor("xh", [128, NFULL, T, BS], mybir.dt.bfloat16,
                        kind="ExternalInput").ap()
    xl = nc.dram_tensor("xl", [128, NFULL, T, BS], mybir.dt.bfloat16,
                        kind="ExternalInput").ap()
    xr = nc.dram_tensor("xr", [KREM, T, BS], mybir.dt.bfloat16,
                        kind="ExternalInput").ap()
    wh = nc.dram_tensor("wh", [128, NFULL, NH], mybir.dt.bfloat16,
                        kind="ExternalInput").ap()
    wl = nc.dram_tensor("wl", [128, NFULL, NH], mybir.dt.bfloat16,
                        kind="ExternalInput").ap()
    wrem = nc.dram_tensor("wrem", [KREM, NH], mybir.dt.bfloat16,
                          kind="ExternalInput").ap()
    wr1 = nc.dram_tensor("wr1", [NH, NH], mybir.dt.bfloat16,
                         kind="ExternalInput").ap()
    w2 = nc.dram_tensor("w2", [NH, 2, NOUT], mybir.dt.bfloat16,
                        kind="ExternalInput").ap()
    b2e = nc.dram_tensor("b2e", [NOUT, 1], mybir.dt.float32,
                         kind="ExternalInput").ap()
    m2out = nc.dram_tensor("m2out", [NOUT, T, BS], mybir.dt.float32,
                           kind="ExternalOutput").ap()
    s2out = nc.dram_tensor("s2out", [NOUT, T, BS], mybir.dt.bfloat16,
                           kind="ExternalOutput").ap()
    if reps_loop:
        reps = nc.dram_tensor("reps", [1, 1], mybir.dt.int32,
                              kind="ExternalInput").ap()

    blocks = []
    t0 = 0
    while t0 < T:
        tb = min(TB, T - t0)
        blocks.append((t0, tb))
        t0 += tb
    t2b = {}
    for bi, (bt0, btb) in enumerate(blocks):
        for ti in range(btb):
            t2b[bt0 + ti] = (bi, ti)

    with tile.TileContext(nc) as tc:
        with tc.tile_pool(name="wpool", bufs=1) as wpool, \
             tc.tile_pool(name="xpool", bufs=3) as xpool, \
             tc.tile_pool(name="state", bufs=1) as state, \
             tc.tile_pool(name="hist", bufs=2) as hist, \
             tc.tile_pool(name="p1pool", bufs=4, space="PSUM") as p1pool, \
             tc.tile_pool(name="p2pool", bufs=2, space="PSUM") as p2pool:

            # ---- constant weights (loaded once) ----
            wh_t = wpool.tile([128, NFULL, NH], mybir.dt.bfloat16)
            nc.sync.dma_start(wh_t[:], wh[:])
            wl_t = wpool.tile([128, NFULL, NH], mybir.dt.bfloat16)
            nc.sync.dma_start(wl_t[:], wl[:])
            wrem_t = wpool.tile([KREM, NH], mybir.dt.bfloat16)
            nc.sync.dma_start(wrem_t[:], wrem[:])
            wr1_t = wpool.tile([NH, NH], mybir.dt.bfloat16)
            nc.sync.dma_start(wr1_t[:], wr1[:])
            w2_t = wpool.tile([NH, 2, NOUT], mybir.dt.bfloat16)
            nc.sync.dma_start(w2_t[:], w2[:])
            b2e_t = wpool.tile([NOUT, 1], mybir.dt.float32)
            nc.sync.dma_start(b2e_t[:], b2e[:])
            biasm1 = wpool.tile([NH, 1], mybir.dt.float32)
            nc.gpsimd.memset(biasm1[:], -THR)

            if reps_loop:
                rt = wpool.tile([1, 1], mybir.dt.int32)
                nc.sync.dma_start(rt[:], reps[:])
                regs = []
                for eng in (nc.tensor, nc.vector, nc.scalar, nc.gpsimd, nc.sync):
                    r = eng.alloc_register(f"reps_{len(regs)}")
                    eng.reg_load(r, rt[0:1, 0:1])
                    regs.append(r)
                rv = nc.snap(bass.RegisterHandles(regs), min_val=0,
                             max_val=1 << 20)
                loop_cm = tc.For_i(0, rv, 1)
            else:
                loop_cm = contextlib.nullcontext()

            with loop_cm:
                # ---- initial state ----
                sign1_init = state.tile([NH, BS], mybir.dt.bfloat16)
                nc.gpsimd.memset(sign1_init[:], -1.0)   # spk1_prev = 0
                mem1_init = state.tile([NH, BS], mybir.dt.float32)
                nc.gpsimd.memset(mem1_init[:], 0.0)
                m2_init = state.tile([NOUT, BS], mybir.dt.float32)
                nc.gpsimd.memset(m2_init[:], 0.0)
                s2_init = state.tile([NOUT, BS], mybir.dt.bfloat16)
                nc.gpsimd.memset(s2_init[:], 0.0)  # spk2_prev = 0

                # ---- per-block input tiles, DMA'd ahead ----
                xh_tiles = [None] * len(blocks)
                xl_tiles = [None] * len(blocks)
                rem_tiles = [None] * len(blocks)

                def load_block(bi):
                    bt0, btb = blocks[bi]
                    xh_b = xpool.tile([128, NFULL, btb, BS], mybir.dt.bfloat16,
                                      name=f"xh_b{bi}", tag="xh_b")
                    xl_b = xpool.tile([128, NFULL, btb, BS], mybir.dt.bfloat16,
                                      name=f"xl_b{bi}", tag="xl_b")
                    rem_b = xpool.tile([KREM, btb, BS], mybir.dt.bfloat16,
                                       name=f"rem_b{bi}", tag="rem_b")
                    # time-halved DMAs: PE can start on the first half while
                    # the rest streams in (matmul order matches arrival order)
                    th = max(btb // 2, 1)
                    nc.sync.dma_start(xh_b[:, :, 0:th, :],
                                      xh[:, :, bt0:bt0 + th, :])
                    nc.sync.dma_start(xl_b[:, :, 0:th, :],
                                      xl[:, :, bt0:bt0 + th, :])
                    nc.sync.dma_start(rem_b[:, 0:th, :],
                                      xr[:, bt0:bt0 + th, :])
                    if th < btb:
                        nc.sync.dma_start(xh_b[:, :, th:btb, :],
                                          xh[:, :, bt0 + th:bt0 + btb, :])
                        nc.sync.dma_start(xl_b[:, :, th:btb, :],
                                          xl[:, :, bt0 + th:bt0 + btb, :])
                        nc.sync.dma_start(rem_b[:, th:btb, :],
                                          xr[:, bt0 + th:bt0 + btb, :])
                    xh_tiles[bi] = xh_b
                    xl_tiles[bi] = xl_b
                    rem_tiles[bi] = rem_b

                def chunks_mms(t, p1):
                    """The 18 full-chunk matmuls for step t (bank opener)."""
                    bi, ti = t2b[t]
                    xh_b, xl_b = xh_tiles[bi], xl_tiles[bi]
                    for c in range(NFULL):
                        nc.tensor.matmul(p1[:], wh_t[:, c, :], xh_b[:, c, ti, :],
                                         start=(c == 0), stop=False)
                    for c in range(NFULL):
                        nc.tensor.matmul(p1[:], wl_t[:, c, :], xh_b[:, c, ti, :],
                                         start=False, stop=False)
                    for c in range(NFULL):
                        nc.tensor.matmul(p1[:], wh_t[:, c, :], xl_b[:, c, ti, :],
                                         start=False, stop=False)

                def rem_mm(t, p1):
                    """Merged-remainder matmul, deferred one step for DMA slack."""
                    bi, ti = t2b[t]
                    nc.tensor.matmul(p1[:], wrem_t[:], rem_tiles[bi][:, ti, :],
                                     start=False, stop=False)

                # ---- software-pipelined main loop ----
                load_block(0)
                load_block(1)

                LOOKAHEAD = 2
                p1_tiles = {}
                for t in range(LOOKAHEAD):
                    p1_tiles[t] = p1pool.tile([NH, BS], mybir.dt.float32,
                                              name=f"p1_{t}", tag="p1")
                    chunks_mms(t, p1_tiles[t])
                rem_mm(0, p1_tiles[0])

                sign1_prev = sign1_init
                mem1_prev = mem1_init
                m2hist_prev, m2pcol = m2_init, 0      # tile + col index of mem2(t-1)
                s2hist_prev, s2pcol = s2_init, 0      # tile + col of spk2(t-1)
                m2hist = s2hist = None

                for t in range(T):
                    bi, ti = t2b[t]
                    bt0, btb = blocks[bi]

                    if ti == 0:
                        # new block: allocate output history tiles
                        m2hist = hist.tile([NOUT, btb * BS], mybir.dt.float32,
                                           name=f"m2h_{bi}", tag="m2h")
                        s2hist = hist.tile([NOUT, btb * BS],
                                           mybir.dt.bfloat16,
                                           name=f"s2h_{bi}", tag="s2h")
                        # prefetch a future block's inputs
                        if bi + 2 < len(blocks):
                            load_block(bi + 2)

                    p1 = p1_tiles.pop(t)
                    # close P1(t): reset matmul  p1 += (-0.5 I) @ sign1(t-1)
                    nc.tensor.matmul(p1[:], wr1_t[:], sign1_prev[:],
                                     start=False, stop=True)
                    # mem1(t) = 0.9*mem1(t-1) + P1(t)   (DVE, psum operand)
                    mem1 = state.tile([NH, BS], mybir.dt.float32,
                                      name=f"mem1_{t % 2}", tag="mem1", bufs=2)
                    nc.vector.scalar_tensor_tensor(
                        mem1[:], mem1_prev[:], BETA, p1[:],
                        mybir.AluOpType.mult, mybir.AluOpType.add)

                    # sign1(t) = Sign(mem1 - 1)  (ACT, bf16 out)
                    sign1 = state.tile([NH, BS], mybir.dt.bfloat16,
                                       name=f"sign1_{t % 3}", tag="sign1", bufs=3)
                    nc.scalar.sign(sign1[:], mem1[:], bias=biasm1[:])

                    # keep TE busy while DVE/ACT run: stream future chunks
                    if t + LOOKAHEAD < T:
                        p1n = p1pool.tile([NH, BS], mybir.dt.float32,
                                          name=f"p1_{t + LOOKAHEAD}", tag="p1")
                        p1_tiles[t + LOOKAHEAD] = p1n
                        chunks_mms(t + LOOKAHEAD, p1n)
                    # deferred remainder matmul for the NEXT step's bank
                    if t + 1 < T:
                        rem_mm(t + 1, p1_tiles[t + 1])

                    # layer 2: P2 = 0.5*W2@sign1 (hi+lo)
                    p2 = p2pool.tile([NOUT, BS], mybir.dt.float32,
                                     name=f"p2_{t % 2}", tag="p2")
                    nc.tensor.matmul(p2[:], w2_t[:, 0, :], sign1[:],
                                     start=True, stop=False)
                    nc.tensor.matmul(p2[:], w2_t[:, 1, :], sign1[:],
                                     start=False, stop=True)

                    # u2 = spk2(t-1) - b2_eff - P2      (DVE, psum operand)
                    u2 = state.tile([NOUT, BS], mybir.dt.float32,
                                    name=f"u2_{t % 2}", tag="u2", bufs=2)
                    nc.vector.scalar_tensor_tensor(
                        u2[:], s2hist_prev[:, s2pcol * BS:(s2pcol + 1) * BS],
                        b2e_t[:, 0:1], p2[:],
                        mybir.AluOpType.subtract, mybir.AluOpType.subtract)
                    # mem2(t) = 0.9*mem2(t-1) - u2 -> written into history col
                    m2dst = m2hist[:, ti * BS:(ti + 1) * BS]
                    nc.vector.scalar_tensor_tensor(
                        m2dst, m2hist_prev[:, m2pcol * BS:(m2pcol + 1) * BS],
                        BETA, u2[:],
                        mybir.AluOpType.mult, mybir.AluOpType.subtract)
                    # spk2(t) = mem2 > 1 (bf16 0/1) -> history col
                    nc.vector.tensor_scalar(
                        s2hist[0:NOUT, ti * BS:(ti + 1) * BS], m2dst, THR, None,
                        mybir.AluOpType.is_gt)

                    mem1_prev = mem1
                    sign1_prev = sign1
                    m2hist_prev, m2pcol = m2hist, ti
                    s2hist_prev, s2pcol = s2hist, ti

                    # store outputs at half-block granularity (ACT HWDGE ring)
                    th = max(btb // 2, 1)
                    if ti == th - 1 and btb > 1:
                        nc.scalar.dma_start(
                            m2out[:, bt0:bt0 + th, :],
                            m2hist[:, 0:th * BS]
                            .rearrange("o (t b) -> o t b", t=th))
                        nc.scalar.dma_start(
                            s2out[:, bt0:bt0 + th, :],
                            s2hist[0:NOUT, 0:th * BS]
                            .rearrange("o (t b) -> o t b", t=th))
                    elif ti == btb - 1:
                        lo = th * BS if btb > 1 else 0
                        tlo = bt0 + th if btb > 1 else bt0
                        nt = bt0 + btb - tlo
                        nc.scalar.dma_start(
                            m2out[:, tlo:bt0 + btb, :],
                            m2hist[:, lo:btb * BS]
                            .rearrange("o (t b) -> o t b", t=nt))
                        nc.scalar.dma_start(
                            s2out[:, tlo:bt0 + btb, :],
                            s2hist[0:NOUT, lo:btb * BS]
                            .rearrange("o (t b) -> o t b", t=nt))

    nc.compile()
    return nc


def prepare_inputs(x, W1, b1, W2, b2):
    """Host-side sharding + dtype splitting. Returns in_maps for 8 cores."""
    x = np.ascontiguousarray(x, dtype=np.float32)
    W1 = np.asarray(W1, dtype=np.float32)
    b1 = np.asarray(b1, dtype=np.float32)
    W2 = np.asarray(W2, dtype=np.float32)
    b2 = np.asarray(b2, dtype=np.float32)

    # hi/lo split of x (bf16), via bit ops (ml_dtypes' bf16->f32 cast is slow)
    u = x.view(np.uint32)
    r = (u + np.uint32(0x7FFF) + ((u >> np.uint32(16)) & np.uint32(1))) \
        & np.uint32(0xFFFF0000)          # round-to-nearest-even to bf16
    xh16 = (r >> np.uint32(16)).astype(np.uint16).view(BF16)
    xl16 = (x - r.view(np.float32)).astype(BF16)

    # W1 splits, transposed to [784, 128]
    W1h = W1.astype(BF16)
    W1l = (W1 - W1h.astype(np.float32)).astype(BF16)
    W1hT = np.ascontiguousarray(W1h.T)
    W1lT = np.ascontiguousarray(W1l.T)
    wh = np.ascontiguousarray(
        W1hT[:NFULL * 128].reshape(NFULL, 128, NH).transpose(1, 0, 2))
    wl = np.ascontiguousarray(
        W1lT[:NFULL * 128].reshape(NFULL, 128, NH).transpose(1, 0, 2))

    # merged remainder weights [KREM, 128]
    b1h, b1m, b1l = _split3_f64(b1.astype(np.float64) - 0.5)
    wrem = np.concatenate([
        W1hT[NFULL * 128:],
        b1h[None, :].astype(BF16), b1m[None, :].astype(BF16),
        b1l[None, :].astype(BF16),
        W1lT[NFULL * 128:],
        W1hT[NFULL * 128:],
    ], axis=0)
    assert wrem.shape == (KREM, NH)

    # layer-1 reset weight: -0.5 * I (exact in bf16)
    wr1 = (-0.5 * np.eye(NH, dtype=np.float32)).astype(BF16)

    W2half = 0.5 * W2.astype(np.float64)        # exact (power of two)
    W2hi = W2half.astype(BF16)
    W2lo = (W2half - W2hi.astype(np.float64)).astype(BF16)
    w2 = np.stack([np.ascontiguousarray(W2hi.T), np.ascontiguousarray(W2lo.T)],
                  axis=1)                        # [128, 2, 10]

    b2eff = (b2.astype(np.float64) + W2half.sum(axis=1)).astype(np.float32)
    b2e = np.ascontiguousarray(b2eff[:, None])   # [10, 1] f32

    in_maps = []
    for c in range(N_CORES):
        sl = slice(c * BS, (c + 1) * BS)
        # packed planes: [128, 6, T, 256] chunk-major
        xh_full = xh16[sl].transpose(2, 1, 0)                    # [784,T,256]
        xl_full = xl16[sl].transpose(2, 1, 0)
        xh_c = np.ascontiguousarray(
            xh_full[:NFULL * 128].reshape(NFULL, 128, T, BS)
            .transpose(1, 0, 2, 3))
        xl_c = np.ascontiguousarray(
            xl_full[:NFULL * 128].reshape(NFULL, 128, T, BS)
            .transpose(1, 0, 2, 3))
        # merged remainder plane [51, T, 256]:
        # rows: xh_rem+ones(19) | xh_rem(16) | xl_rem(16)
        xr_c = np.empty((KREM, T, BS), BF16)
        xr_c[0:REM] = xh_full[NFULL * 128:]
        xr_c[REM:REM + 3] = 1.0
        xr_c[REM + 3:2 * REM + 3] = xh_full[NFULL * 128:]
        xr_c[2 * REM + 3:] = xl_full[NFULL * 128:]
        in_maps.append({
            "xh": xh_c, "xl": xl_c, "xr": xr_c, "wh": wh, "wl": wl,
            "wrem": wrem, "wr1": wr1, "w2": w2, "b2e": b2e,
        })
    return in_maps


def postprocess(results):
    """Gather per-core outputs into (spk2_rec, mem2_rec) [T, B, 10] f32."""
    spk = np.empty((T, B, NOUT), np.float32)
    mem = np.empty((T, B, NOUT), np.float32)
    for c, r in enumerate(results):
        sl = slice(c * BS, (c + 1) * BS)
        mem[:, sl, :] = r["m2out"].transpose(1, 2, 0)
        spk[:, sl, :] = r["s2out"].astype(np.float32).transpose(1, 2, 0)
    return spk, mem


_NC_CACHE = {}


def kernel(x, W1, b1, W2, b2):
    if "nc" not in _NC_CACHE:
        _NC_CACHE["nc"] = build_kernel(reps_loop=False)
    nc = _NC_CACHE["nc"]
    in_maps = prepare_inputs(x, W1, b1, W2, b2)
    res = bass_utils.run_bass_kernel_spmd(
        nc, in_maps, core_ids=list(range(N_CORES)))
    return postprocess(res.results)
